# revision 12
# baseline (speedup 1.0000x reference)
"""DeepSeek-MoE layer on 8 Trainium2 NeuronCores (expert-parallel, sparse dispatch).

Sharding:
  - Expert-parallel: core c owns experts [4c, 4c+4). The expert axis is
    permuted per core in the staged inputs so every core's LOCAL experts are
    logits columns 0..3 (keeps the SPMD program core-independent).
  - Shared expert: sharded on the intermediate dim (core c owns a 1024-wide
    slice of I=8192); partial [T, H] outputs summed on host.
  - Router + x replicated; router logits output taken from core 0.

Device program per core:
  1. Router logits [E, T] in true fp32 (top-4 selection is gap-sensitive:
     min 4th/5th logit gap in-distribution is ~5e-5, so reduced-precision
     matmuls would flip expert selections).
  2. Per 128-token tile: DVE max8 -> top-4 threshold + softmax combine
     weights cw in fp32 (matches jax top_k + softmax for distinct logits).
  3. Shared-expert FFN (fp16, fp32 accumulation) over all tokens for the
     local I-slice; x kept resident in an h-major fp16 layout so every
     moving operand is contiguous.
  4. Per local expert: GPSIMD sparse_gather compacts routed token ids
     (capacity C=384 >= observed max 281, tail-padded with out-of-range ids
     so padded slots self-identify); per-h-block ap_gather pulls token
     columns straight into matmul-ready tiles; bounced via DRAM. Runs on
     GPSIMD/DVE/DMA concurrently with the shared-expert matmuls.
  5. Expert FFNs over gathered tokens only (fp16, fp32 accumulation), with
     the expert's full down-projection weight resident so each stationary
     tile serves 4 moving matmuls; per-slot combine weights applied in fp32
     on the down-proj output via per-partition scalars.

Host: scatter-add slot outputs to token rows (the all-to-all return), sum
per-core shared partials, unpermute core 0's logits.
"""

import numpy as np

import concourse.bacc as bacc
import concourse.mybir as mybir
import concourse.tile as tile
from concourse.bass_utils import run_bass_kernel_spmd
from concourse.masks import make_identity

F32 = mybir.dt.float32
F16 = mybir.dt.float16
I16 = mybir.dt.int16
U32 = mybir.dt.uint32
AF = mybir.ActivationFunctionType
ALU = mybir.AluOpType
AXX = mybir.AxisListType.X

N_CORES = 8
B, S, H, E, K, M, I = 2, 1024, 2048, 32, 4, 1408, 8192
T = B * S                      # 2048 tokens
HB = H // 128                  # 16 h blocks
EL = E // N_CORES              # 4 local experts
MT = M // 128                  # 11 m tiles
ISH = I // N_CORES             # 1024 shared-intermediate slice
IT = ISH // 128                # 8 i tiles
C = 384                        # per-expert token capacity (max observed 281)
CW = C // 16
TW = T // 16                   # 128
NCH = 4                        # 512-token chunks
MGROUPS = ((0, 4), (4, 8), (8, 11))

_CACHE = {}


def _build_nc():
    nc = bacc.Bacc("TRN2", target_bir_lowering=False, debug=False,
                   num_devices=N_CORES)

    xtbr = nc.dram_tensor("xtbr", [HB, 128, T], F16, kind="ExternalInput")
    xtbf = nc.dram_tensor("xtbf", [HB, 128, T], F32, kind="ExternalInput")
    gwb = nc.dram_tensor("gwb", [128, HB, E], F32, kind="ExternalInput")
    w1l = nc.dram_tensor("w1l", [EL, HB, 128, M], F16, kind="ExternalInput")
    w2l = nc.dram_tensor("w2l", [EL, MT, 128, H], F16, kind="ExternalInput")
    ws1c = nc.dram_tensor("ws1c", [HB, 128, ISH], F16, kind="ExternalInput")
    ws2c = nc.dram_tensor("ws2c", [IT, 128, H], F16, kind="ExternalInput")
    iotap1 = nc.dram_tensor("iotap1", [16, TW + CW], F32, kind="ExternalInput")

    logits_out = nc.dram_tensor("logits_out", [T, E], F32, kind="ExternalOutput")
    shared_out = nc.dram_tensor("shared_out", [T, H], F32, kind="ExternalOutput")
    slots_out = nc.dram_tensor("slots_out", [EL, C, H], F32, kind="ExternalOutput")
    tok_out = nc.dram_tensor("tok_out", [EL, 16, CW], F32, kind="ExternalOutput")

    cwT_dram = nc.dram_tensor("cwT_dram", [E, T], F32)
    xe_dram = nc.dram_tensor("xe_dram", [EL, 128, HB, C], F16)

    with tile.TileContext(nc) as tc:
        with tc.tile_pool(name="pp", bufs=1) as pp:
            ident = pp.tile([128, 128], F32)
            make_identity(nc, ident[:])
            iop1 = pp.tile([16, TW + CW], F32)
            nc.sync.dma_start(iop1[:], iotap1[:])
            cwl = pp.tile([128, EL, 3], F32)

            # x resident in h-major fp16: [128, hb, T]
            with tc.tile_pool(name="xp", bufs=1) as xp:
                x2 = xp.tile([128, HB, T], F16)
                for hb in range(HB):
                    nc.sync.dma_start(x2[:, hb, :], xtbr[hb])

                # ============ router + cw (true fp32, x streamed) ============
                with tc.tile_pool(name="rp", bufs=1) as rp, \
                     tc.tile_pool(name="rw", bufs=2) as rw, \
                     tc.tile_pool(name="rx", bufs=6) as rx, \
                     tc.tile_pool(name="ps1", bufs=1, space="PSUM") as ps1:
                    g_sb = rp.tile([128, HB, E], F32)
                    nc.sync.dma_start(g_sb[:], gwb[:])
                    lsb = rp.tile([E, T], F32)
                    for ch in range(NCH):
                        pt = ps1.tile([E, 512], F32, tag="rps")
                        for hb in range(HB):
                            xrt = rx.tile([128, 512], F32, tag="xrt")
                            nc.sync.dma_start(
                                xrt[:], xtbf[hb, :, ch * 512:(ch + 1) * 512])
                            nc.tensor.matmul(
                                pt[:], lhsT=g_sb[:, hb, :], rhs=xrt[:],
                                start=(hb == 0), stop=(hb == HB - 1))
                        nc.vector.tensor_copy(lsb[:, ch * 512:(ch + 1) * 512],
                                              pt[:])

                    cwT = rp.tile([E, T], F32)
                    for j in range(T // 128):
                        ptt = ps1.tile([128, E], F32, tag="tps")
                        nc.tensor.transpose(
                            ptt[:], lsb[:, j * 128:(j + 1) * 128], ident[:E, :E])
                        lf = rw.tile([128, E], F32, tag="lf")
                        nc.vector.tensor_copy(lf[:], ptt[:])
                        nc.sync.dma_start(logits_out[j * 128:(j + 1) * 128, :],
                                          lf[:])

                        m8 = rw.tile([128, 8], F32, tag="m8")
                        nc.vector.max(out=m8[:], in_=lf[:])
                        negmax = rw.tile([128, 1], F32, tag="negmax")
                        nc.vector.tensor_scalar_mul(negmax[:], m8[:, 0:1], -1.0)
                        exp4 = rw.tile([128, 4], F32, tag="exp4")
                        nc.scalar.activation(exp4[:], m8[:, 0:4], AF.Exp,
                                             bias=negmax[:])
                        den = rw.tile([128, 1], F32, tag="den")
                        nc.vector.reduce_sum(den[:], exp4[:], axis=AXX)
                        rden = rw.tile([128, 1], F32, tag="rden")
                        nc.vector.reciprocal(rden[:], den[:])

                        mask = rw.tile([128, E], F32, tag="mask")
                        nc.vector.tensor_tensor(
                            mask[:], lf[:], m8[:, 3:4].to_broadcast([128, E]),
                            op=ALU.is_ge)
                        cwt = rw.tile([128, E], F32, tag="cwt")
                        nc.scalar.activation(cwt[:], lf[:], AF.Exp,
                                             bias=negmax[:])
                        nc.vector.tensor_mul(cwt[:], cwt[:], mask[:])
                        nc.vector.tensor_scalar_mul(cwt[:], cwt[:], rden[:])

                        ptc = ps1.tile([E, 128], F32, tag="tpsb")
                        nc.tensor.transpose(ptc[:], cwt[:], ident[:])
                        nc.vector.tensor_copy(cwT[:, j * 128:(j + 1) * 128],
                                              ptc[:])
                    nc.sync.dma_start(cwT_dram[:], cwT[:])

                # ============ shared expert up (fp16, contiguous rhs) ========
                with tc.tile_pool(name="shp", bufs=6) as shp, \
                     tc.tile_pool(name="shb", bufs=1) as shb:
                    s_sb = shb.tile([128, IT, T], F16)
                    with tc.tile_pool(name="psu", bufs=1, space="PSUM") as psu:
                        for half in range(2):
                            t0 = half * 1024
                            for g2 in range(4):
                                pts = [psu.tile([128, 512], F32,
                                                tag=f"su{q}", name=f"su{q}")
                                       for q in range(4)]
                                for hb in range(HB):
                                    wt = shp.tile([128, 256], F16, tag="ws1t")
                                    nc.sync.dma_start(
                                        wt[:],
                                        ws1c[hb, :, g2 * 256:(g2 + 1) * 256])
                                    for q in range(4):
                                        it, c2 = g2 * 2 + q // 2, q % 2
                                        nc.tensor.matmul(
                                            pts[q][:],
                                            lhsT=wt[:, (q // 2) * 128:
                                                    (q // 2 + 1) * 128],
                                            rhs=x2[:, hb, t0 + c2 * 512:
                                                   t0 + (c2 + 1) * 512],
                                            start=(hb == 0),
                                            stop=(hb == HB - 1))
                                for q in range(4):
                                    it, c2 = g2 * 2 + q // 2, q % 2
                                    nc.scalar.activation(
                                        s_sb[:, it, t0 + c2 * 512:
                                             t0 + (c2 + 1) * 512],
                                        pts[q][:], AF.Silu)

                    # ======== dispatch (GPSIMD/DVE/DMA; overlaps shared) =====
                    with tc.tile_pool(name="dw", bufs=2) as dw, \
                         tc.tile_pool(name="db", bufs=1) as db, \
                         tc.tile_pool(name="ps2", bufs=2, space="PSUM") as ps2:
                        idx_all = db.tile([128, EL, CW], I16)
                        for e in range(EL):
                            v = dw.tile([128, 16], F32, tag="vrow")
                            nc.sync.dma_start(
                                v[:],
                                cwT_dram[e].rearrange("(f q) -> f q", q=16))
                            wps = ps2.tile([16, 128], F32, tag="wps")
                            nc.tensor.transpose(wps[:], v[:], ident[:])
                            wext = dw.tile([16, TW + CW], F32, tag="wext")
                            nc.vector.tensor_copy(wext[:, :TW], wps[:])
                            nc.vector.memset(wext[:, TW:], 1.0)

                            ge0 = dw.tile([16, TW + CW], F32, tag="ge0")
                            nc.vector.tensor_scalar(ge0[:], wext[:], 0.0, None,
                                                    op0=ALU.is_gt)
                            sgt = dw.tile([16, TW + CW], F32, tag="sgt")
                            nc.vector.tensor_mul(sgt[:], ge0[:], iop1[:])
                            nc.vector.tensor_scalar_add(sgt[:], sgt[:], -1.0)

                            tok_sl = dw.tile([16, CW], F32, tag="toksl")
                            nfound = dw.tile([1, 1], U32, tag="nf")
                            nc.gpsimd.sparse_gather(tok_sl[:], sgt[:],
                                                    num_found=nfound[:])
                            nc.sync.dma_start(tok_out[e], tok_sl[:])

                            tok_cl = dw.tile([16, CW], F32, tag="tokcl")
                            nc.vector.tensor_scalar_min(tok_cl[:], tok_sl[:],
                                                        float(T - 1))
                            tok_i16 = dw.tile([16, CW], I16, tag="toki")
                            nc.vector.tensor_copy(tok_i16[:], tok_cl[:])
                            for k in range(8):
                                nc.sync.dma_start(
                                    idx_all[k * 16:(k + 1) * 16, e, :],
                                    tok_i16[:])

                            # cw by slot -> per-partition slot scalars
                            cwrep = db.tile([128, T], F32, tag="cwrep")
                            nc.sync.dma_start(cwrep[0:1, :],
                                              cwT_dram[e][None, :])
                            p = 1
                            while p < 128:
                                nc.sync.dma_start(cwrep[p:2 * p, :],
                                                  cwrep[0:p, :])
                                p *= 2
                            cwga = dw.tile([128, C], F32, tag="cwga")
                            nc.gpsimd.ap_gather(cwga[:], cwrep[:],
                                                idx_all[:, e, :],
                                                channels=128, num_elems=T,
                                                d=1, num_idxs=C)
                            for st in range(3):
                                cps = ps2.tile([128, 128], F32, tag="cps")
                                nc.tensor.transpose(
                                    cps[:], cwga[:, st * 128:(st + 1) * 128],
                                    ident[:])
                                nc.vector.tensor_copy(cwl[:, e, st:st + 1],
                                                      cps[:, 0:1])

                        # token gathers: stream fp32 x per h-block, gather all
                        # experts, cast to fp16 stagings, bounce via DRAM
                        for hq in range(4):
                            stgs = [dw.tile([128, 4, C], F16, tag=f"stg{e}",
                                            name=f"stg{e}")
                                    for e in range(EL)]
                            for h2 in range(4):
                                hb = hq * 4 + h2
                                xf = dw.tile([128, T], F32, tag="xf")
                                nc.sync.dma_start(xf[:], xtbf[hb])
                                for e in range(EL):
                                    graw = dw.tile([128, C], F32, tag="graw")
                                    nc.gpsimd.ap_gather(
                                        graw[:], xf[:], idx_all[:, e, :],
                                        channels=128, num_elems=T, d=1,
                                        num_idxs=C)
                                    nc.vector.tensor_copy(stgs[e][:, h2, :],
                                                          graw[:])
                            for e in range(EL):
                                nc.sync.dma_start(
                                    xe_dram[e][:, hq * 4:(hq + 1) * 4, :],
                                    stgs[e][:])

            # ============ shared expert down (x freed) ============
                    with tc.tile_pool(name="psd", bufs=1, space="PSUM") as psd:
                        ws2res = shb.tile([128, IT, H], F16)
                        for it in range(IT):
                            nc.sync.dma_start(ws2res[:, it, :], ws2c[it])
                        for t_ in range(16):
                            pods = [psd.tile([128, 512], F32,
                                             tag=f"sd{q}", name=f"sd{q}")
                                    for q in range(4)]
                            for it in range(IT):
                                for hs in range(4):
                                    nc.tensor.matmul(
                                        pods[hs][:],
                                        lhsT=s_sb[:, it,
                                                  t_ * 128:(t_ + 1) * 128],
                                        rhs=ws2res[:, it,
                                                   hs * 512:(hs + 1) * 512],
                                        start=(it == 0), stop=(it == IT - 1))
                            for hs in range(4):
                                so = shp.tile([128, 512], F32, tag="so")
                                nc.vector.tensor_copy(so[:], pods[hs][:])
                                nc.sync.dma_start(
                                    shared_out[t_ * 128:(t_ + 1) * 128,
                                               hs * 512:(hs + 1) * 512],
                                    so[:])

            # ============ expert FFNs over gathered tokens (fp16) ============
            with tc.tile_pool(name="ep", bufs=2) as ep, \
                 tc.tile_pool(name="ew", bufs=6) as ew:
                for e in range(EL):
                    xe = ep.tile([128, HB, C], F16, tag="xe")
                    for hq in range(4):
                        nc.sync.dma_start(
                            xe[:, hq * 4:(hq + 1) * 4, :],
                            xe_dram[e][:, hq * 4:(hq + 1) * 4, :])
                    a_sb = ep.tile([128, MT, C], F16, tag="a_sb")
                    w2res = ep.tile([128, MT, H], F16, tag="w2res")
                    for m in range(MT):
                        nc.sync.dma_start(w2res[:, m, :], w2l[e, m])
                    with tc.tile_pool(name="pse", bufs=1, space="PSUM") as pse:
                        for g0, g1 in MGROUPS:
                            pas = [pse.tile([128, C], F32, tag=f"pa{m - g0}",
                                            name=f"pa{m - g0}")
                                   for m in range(g0, g1)]
                            for hb in range(HB):
                                w1t = ew.tile([128, 512], F16, tag="w1t")
                                nc.sync.dma_start(
                                    w1t[:, :(g1 - g0) * 128],
                                    w1l[e, hb, :, g0 * 128:g1 * 128])
                                for m in range(g0, g1):
                                    nc.tensor.matmul(
                                        pas[m - g0][:],
                                        lhsT=w1t[:, (m - g0) * 128:
                                                 (m - g0 + 1) * 128],
                                        rhs=xe[:, hb, :],
                                        start=(hb == 0), stop=(hb == HB - 1))
                            for m in range(g0, g1):
                                nc.scalar.activation(
                                    a_sb[:, m, :], pas[m - g0][:], AF.Silu)
                    with tc.tile_pool(name="psf", bufs=1, space="PSUM") as psf:
                        for st in range(3):
                            pos = [psf.tile([128, 512], F32, tag=f"po{hs}",
                                            name=f"po{hs}")
                                   for hs in range(4)]
                            for m in range(MT):
                                for hs in range(4):
                                    nc.tensor.matmul(
                                        pos[hs][:],
                                        lhsT=a_sb[:, m,
                                                  st * 128:(st + 1) * 128],
                                        rhs=w2res[:, m,
                                                  hs * 512:(hs + 1) * 512],
                                        start=(m == 0), stop=(m == MT - 1))
                            for hs in range(4):
                                so = ew.tile([128, 512], F32, tag="eso")
                                nc.vector.tensor_scalar_mul(
                                    so[:], pos[hs][:], cwl[:, e, st:st + 1])
                                nc.sync.dma_start(
                                    slots_out[e, st * 128:(st + 1) * 128,
                                              hs * 512:(hs + 1) * 512], so[:])
    nc.compile()
    return nc


def _stage_inputs(hidden_states, gate_w, w1, w2, ws1, ws2):
    x = np.ascontiguousarray(hidden_states.reshape(T, H), dtype=np.float32)
    xhm = x.reshape(T, HB, 128).transpose(1, 2, 0)          # [HB, 128, T]
    xtbr = np.ascontiguousarray(xhm.astype(np.float16))
    xtbf = np.ascontiguousarray(xhm)

    body = (np.arange(T, dtype=np.float32) + 1.0).reshape(TW, 16).T
    tail = (T + np.arange(C, dtype=np.float32) + 1.0).reshape(CW, 16).T
    iotap1 = np.ascontiguousarray(np.concatenate([body, tail], axis=1))

    in_maps = []
    perms = []
    for c in range(N_CORES):
        local = list(range(EL * c, EL * (c + 1)))
        rest = [e for e in range(E) if e not in local]
        perm = np.array(local + rest, dtype=np.int64)
        perms.append(perm)
        gwp = np.ascontiguousarray(gate_w[:, perm], dtype=np.float32)
        gwb = np.ascontiguousarray(gwp.reshape(HB, 128, E).transpose(1, 0, 2))
        w1c = np.ascontiguousarray(
            w1[local].reshape(EL, HB, 128, M).astype(np.float16))
        w2c = np.ascontiguousarray(
            w2[local].reshape(EL, MT, 128, H).astype(np.float16))
        ws1cc = np.ascontiguousarray(
            ws1[:, ISH * c:ISH * (c + 1)].reshape(HB, 128, ISH)
            .astype(np.float16))
        ws2cc = np.ascontiguousarray(
            ws2[ISH * c:ISH * (c + 1)].reshape(IT, 128, H).astype(np.float16))
        in_maps.append({
            "xtbr": xtbr, "xtbf": xtbf, "gwb": gwb, "w1l": w1c, "w2l": w2c,
            "ws1c": ws1cc, "ws2c": ws2cc, "iotap1": iotap1,
        })
    return in_maps, perms


def run_cores(in_maps, **kwargs):
    if "nc" not in _CACHE:
        _CACHE["nc"] = _build_nc()
    return run_bass_kernel_spmd(
        _CACHE["nc"], in_maps, list(range(N_CORES)), **kwargs)


def combine(results, perms):
    out = np.zeros((T, H), dtype=np.float32)
    for c in range(N_CORES):
        out += results[c]["shared_out"]
    for c in range(N_CORES):
        slots = results[c]["slots_out"]          # [EL, C, H]
        toks = results[c]["tok_out"]             # [EL, 16, CW] fp32
        for e in range(EL):
            tok = toks[e].T.reshape(-1).astype(np.int64)  # unwrap
            valid = tok < T
            out[tok[valid]] += slots[e][valid]
    inv0 = np.argsort(perms[0])
    logits = results[0]["logits_out"][:, inv0]
    return (np.ascontiguousarray(out.reshape(B, S, H)),
            np.ascontiguousarray(logits.reshape(B, S, E)))


def kernel(**inputs):
    in_maps, perms = _stage_inputs(
        np.asarray(inputs["hidden_states"]), np.asarray(inputs["gate_w"]),
        np.asarray(inputs["w1"]), np.asarray(inputs["w2"]),
        np.asarray(inputs["ws1"]), np.asarray(inputs["ws2"]))
    res = run_cores(in_maps)
    return combine(res.results, perms)
